# revision 28
# baseline (speedup 1.0000x reference)
"""Trainium2 Bass kernel for: Conv3d(3->16, k=3x3x3, VALID) + bias -> min over
depth -> softmax over channels.

Input  x: (16, 3, 32, 128, 128) f32   [N, C_in, D, H, W]
Weight w: (16, 3, 3, 3, 3) f32        [C_out, C_in, kD, kH, kW]
Bias   b: (16,) f32
Output  : (16, 16, 126, 126) f32      [N, C_out, H_out, W_out]

Data-parallel over batch: 2 batches per core x 8 cores. Per core:

  - x (bf16, host-transposed to [N, D, C, H, W] + one pad row) lives in quad
    tiles: strip r rows 0..11 = (d-window of 4) x (ci 3) for depth pair
    d0=8g+2r, rows 12..23 = the same planes shifted by one h-row.  Each
    strip loads with ONE 3-dim DMA (overlapping reads express the shift).
  - Conv as 16-way tile_position-packed bf16 matmuls, 6 accumulating passes
    per PSUM supertile: 3x K=12 (kh=2 at kw=0..2, base rows only, run first
    so they only wait on rows 0..11) then 3x K=24 (kh=0 and kh=1 fused via
    the shifted copy, kw=0..2).  PSUM supertile [128, 4*512]: partition =
    (c 4 chunks x delta x co), free = (pair r x 512 spatial).
  - Depth-min: g=0 supertile copied PSUM->SBUF on the Scalar engine
    (activation Copy); g=1..3 DVE tensor_tensor(min) against the running
    SBUF buffer.  DVE folds the 4 pair slots pairwise, then the delta
    halves are folded via merged shuffle DMAs (compute engines cannot
    address 16-partition offsets; one DMA per (k,c) moves both delta slots)
    plus one aligned DVE min.
  - Softmax over co: ACT exp (bias fused; min(y)+b == min(y+b)) -> bf16,
    PE ones-matmul for co-sums, DVE fast-approx reciprocal, PE ones-matmul
    partition broadcast, DVE multiply.  The result is dumped to DRAM in its
    native [128, 512] layout (one DMA per (n, hh, cp)); the host does the
    final (k,c,co,i,w) -> (co,h,w) untangling in numpy.
  - Each phase's softmax tail is emitted AFTER the next phase's conv
    matmuls (software pipelining) so the tail's PE ops never stall the
    conv pipeline; DMAs are spread across the Sync/Scalar HWDGE queues and
    the Pool SWDGE queue to dodge the ~2us-per-DMA queue serialization.
"""

import os
import sys

sys.path.insert(0, "/opt/trn_rl_repo")

import numpy as np

import concourse.bass as bass
import concourse.bacc as bacc
import concourse.tile as tile
import concourse.mybir as mybir
from concourse import bass_utils
from concourse.ap import AP

F32 = mybir.dt.float32
BF16 = mybir.dt.bfloat16

N_CORES = 8
NB = 2           # batches per core
CI = 3
D = 32
H = 128
W = 128
CO = 16
CHUNK = 512
NGRP = 4         # pair-quad groups; g<3: 4 pairs, g=3: 3 pairs
NCGL = 4         # chunk groups per h-half (each 4 col-tiles x 512)
HOUT = 126
WOUT = 126
PAD = 320
QF = 66 * W + PAD  # quad tile free size (worst case hh=0)
HW = H * W
XFLAT = NB * D * CI * HW + W  # one pad row so shifted reads never run off

_COMPILED = {}


def _pairs_in_group(g):
    return 4 if g < 3 else 3


def _build_weight_blocks(conv_weight):
    """[128, 192]: 6 col-sets of 32 (delta*16+co). Strip rows 0..11 hold the
    unshifted x copy, rows 12..23 the h+1-shifted copy. Sets 0-2 (K=24)
    contract kh=0 (rows 0-11) and kh=1 (rows 12-23) together at kw=set;
    sets 3-5 (K=12) contract kh=2 at kw=set-3. Value = w[co,ci,dl-delta,
    kh,kw], 0 outside kd range. Replicated per strip."""
    blk = np.zeros((24, 6, 32), dtype=np.float32)
    for dl in range(4):
        for ci in range(CI):
            row = dl * 3 + ci
            for delta in range(2):
                kd = dl - delta
                if not (0 <= kd <= 2):
                    continue
                cs = slice(delta * 16, (delta + 1) * 16)
                for kw in range(3):
                    blk[row, kw, cs] = conv_weight[:, ci, kd, 0, kw]
                    blk[row + 12, kw, cs] = conv_weight[:, ci, kd, 1, kw]
                    blk[row, 3 + kw, cs] = conv_weight[:, ci, kd, 2, kw]
    w_sb = np.zeros((128, 192), dtype=np.float32)
    for r in range(4):
        w_sb[32 * r:32 * r + 24, :] = blk.reshape(24, 192)
    return w_sb


def _build_ones():
    """[128, 8]: col k*4+c sums partitions {k*64 + c*16 + co : co}."""
    ones = np.zeros((128, 8), dtype=np.float32)
    for p in range(128):
        k, c = p // 64, (p % 64) // 16
        ones[p, k * 4 + c] = 1.0
    return ones


def _build_ones_bc():
    """[8, 128]: transpose of _build_ones -- broadcasts row j over its
    16-partition group."""
    return np.ascontiguousarray(_build_ones().T)


def _build_bias128(conv_bias):
    """[128, 1]: partition k*64 + c*16 + co -> bias[co]."""
    b = np.zeros((128, 1), dtype=np.float32)
    for p in range(128):
        b[p, 0] = conv_bias[p % 16]
    return b


def _emit_kernel(tc):
    nc = tc.nc
    x_t = nc.dram_tensor("x", [XFLAT], BF16, kind="ExternalInput")
    w_ap = nc.dram_tensor("w", [128, 192], BF16, kind="ExternalInput").ap()
    bias_ap = nc.dram_tensor("bias", [128, 1], F32, kind="ExternalInput").ap()
    ones_ap = nc.dram_tensor("ones", [128, 8], BF16, kind="ExternalInput").ap()
    ones_bc_ap = nc.dram_tensor("ones_bc", [8, 128], BF16,
                                kind="ExternalInput").ap()
    y_ap = nc.dram_tensor("y", [NB, 2, 2, 128, CHUNK], F32,
                          kind="ExternalOutput").ap()

    from contextlib import ExitStack

    MIN = mybir.AluOpType.min

    with ExitStack() as ctx:
        const_pool = ctx.enter_context(tc.tile_pool(name="const", bufs=1))
        in_pool = ctx.enter_context(tc.tile_pool(name="in", bufs=7))
        run_pool = ctx.enter_context(tc.tile_pool(name="run", bufs=3))
        sm_pool = ctx.enter_context(tc.tile_pool(name="sm", bufs=3))
        exp_pool = ctx.enter_context(tc.tile_pool(name="expp", bufs=3))
        psum_pool = ctx.enter_context(tc.tile_pool(name="ps", bufs=2,
                                                   space="PSUM"))

        w_sb = const_pool.tile([128, 192], BF16, tag="w")
        nc.scalar.dma_start(w_sb[:, :], w_ap[:, :])
        bias_sb = const_pool.tile([128, 1], F32, tag="bias")
        nc.scalar.dma_start(bias_sb[:, :], bias_ap[:, :])
        ones_sb = const_pool.tile([128, 8], BF16, tag="ones")
        nc.scalar.dma_start(ones_sb[:, :], ones_ap[:, :])
        ones_bc_sb = const_pool.tile([8, 128], BF16, tag="onesbc")
        nc.scalar.dma_start(ones_bc_sb[:, :], ones_bc_ap[:, :])


        def emit_tail(n, hh, running):
            # tail for phase (n, hh): fold pair slots (free dim, DVE), then
            # merged delta shuffles into t2 [128, 2*512] (partition p = 64k
            # + 16c + co, free slot = delta), one aligned min, softmax, and
            # a single native-layout dump per cp.  Runs one phase late.
            for cp in range(NCGL // 2):
                t2 = sm_pool.tile([128, 2 * CHUNK], BF16, tag="t2")
                for k in range(2):
                    cgl = 2 * cp + k
                    run = running[cgl]
                    a = sm_pool.tile([128, CHUNK], BF16, tag="a")
                    b = sm_pool.tile([128, CHUNK], BF16, tag="b")
                    nc.vector.tensor_tensor(
                        out=a[:, :], in0=run[:, 0:512],
                        in1=run[:, 512:1024], op=MIN)
                    nc.vector.tensor_tensor(
                        out=b[:, :], in0=run[:, 1024:1536],
                        in1=run[:, 1536:2048], op=MIN)
                    rm = sm_pool.tile([128, CHUNK], BF16, tag="rm")
                    nc.vector.tensor_tensor(
                        out=rm[:, :], in0=a[:, :], in1=b[:, :], op=MIN)
                    engs = (nc.sync, nc.scalar, nc.scalar)
                    for c in range(4):
                        base = 64 * k + 16 * c
                        for dlt in range(2):
                            q = ((2 * cp + k) * 4 + c) * 2 + dlt
                            engs[q % 2].dma_start(
                                t2[base:base + 16,
                                   CHUNK * dlt:CHUNK * (dlt + 1)],
                                rm[32 * c + 16 * dlt:
                                   32 * c + 16 * dlt + 16, :])
                sm = sm_pool.tile([128, CHUNK], BF16, tag="sm")
                nc.vector.tensor_tensor(
                    out=sm[:, :], in0=t2[:, 0:CHUNK],
                    in1=t2[:, CHUNK:2 * CHUNK], op=MIN)
                expt = exp_pool.tile([128, CHUNK], BF16, tag="exp")
                nc.scalar.activation(
                    expt[:, :], sm[:, :],
                    mybir.ActivationFunctionType.Exp,
                    bias=bias_sb[:, :], scale=1.0)
                esum = psum_pool.tile([8, CHUNK], F32, tag="big")
                nc.tensor.matmul(esum[:, :], lhsT=ones_sb[:, :],
                                 rhs=expt[:, :], start=True, stop=True)
                rec = sm_pool.tile([8, CHUNK], F32, tag="rec")
                nc.vector.reciprocal_approx_fast(rec[:, :], esum[:, :])
                rec_bf = sm_pool.tile([8, CHUNK], BF16, tag="recbf")
                nc.vector.tensor_copy(rec_bf[:, :], rec[:, :])
                # broadcast rec rows over their 16-co partition groups
                # via a K=8 ones matmul (PE does the partition fan-out)
                rb = psum_pool.tile([128, CHUNK], F32, tag="big")
                nc.tensor.matmul(rb[:, :], lhsT=ones_bc_sb[:, :],
                                 rhs=rec_bf[:, :], start=True, stop=True)
                soft = sm_pool.tile([128, CHUNK], F32, tag="soft")
                nc.vector.tensor_tensor(
                    out=soft[:, :], in0=rb[:, :], in1=expt[:, :],
                    op=mybir.AluOpType.mult)
                nc.scalar.dma_start(y_ap[n, hh, cp], soft[:, :])

        prev_phase = None
        for n in range(NB):
            for hh in range(2):
                h0 = 64 * hh
                hrows = 66 if hh == 0 else 64
                running = [run_pool.tile([128, 4 * CHUNK], BF16,
                                         name=f"run{cg}", tag=f"run{cg}")
                           for cg in range(NCGL)]
                for g in range(NGRP):
                    npairs = _pairs_in_group(g)
                    quad = in_pool.tile([128, QF], BF16, tag="quad")
                    nc.gpsimd.memset(quad[:, (hrows - 1) * W:QF], 0.0)
                    # One 3-dim DMA per strip loads rows 0..11 (x planes at
                    # h0) AND rows 12..23 (same planes at h0+1): dims =
                    # [h-shift j:2 x step W][plane dc:12 x step HW][inner].
                    # hh=1's shifted copy reads one garbage row past each
                    # plane; it only feeds discarded h=127 outputs, and the
                    # host pads x with one row so the last read stays in
                    # bounds.
                    inner = hrows * W
                    for r in range(npairs):
                        d0 = 8 * g + 2 * r
                        off = (n * D + d0) * CI * HW + h0 * W
                        src = AP(x_t, off, [[W, 2], [HW, 12], [1, inner]])
                        eng = nc.sync if r % 2 == 0 else nc.scalar
                        eng.dma_start(
                            quad[32 * r:32 * r + 24, 0:inner].rearrange(
                                "(j q) f -> j q f", j=2), src)
                    for cgl in range(NCGL):
                        ps = psum_pool.tile([128, 4 * CHUNK], F32, tag="big")
                        for s in (3, 4, 5, 0, 1, 2):
                            kw = s % 3
                            koff = kw if s < 3 else 2 * W + kw
                            kk = 24 if s < 3 else 12
                            for r in range(npairs):
                                for c in range(4):
                                    s0 = cgl * 2048 + c * CHUNK + koff
                                    nc.tensor.matmul(
                                        ps[32 * c:32 * c + 32,
                                           r * CHUNK:(r + 1) * CHUNK],
                                        lhsT=w_sb[32 * r:32 * r + kk,
                                                  s * 32:(s + 1) * 32],
                                        rhs=quad[32 * r:32 * r + kk,
                                                 s0:s0 + CHUNK],
                                        start=(s == 3),
                                        stop=(s == 2),
                                        tile_position=(32 * r, 32 * c),
                                        skip_group_check=True,
                                    )
                        width = npairs * CHUNK
                        if g == 0:
                            nc.vector.tensor_copy(
                                running[cgl][:, 0:width], ps[:, 0:width])
                        else:
                            nc.vector.tensor_tensor(
                                out=running[cgl][:, 0:width],
                                in0=ps[:, 0:width],
                                in1=running[cgl][:, 0:width],
                                op=MIN,
                            )
                if prev_phase is not None:
                    emit_tail(*prev_phase)
                prev_phase = (n, hh, running)
        emit_tail(*prev_phase)


def _compile():
    if "nc" in _COMPILED:
        return _COMPILED["nc"]
    nc = bacc.Bacc("TRN2", target_bir_lowering=False, debug=False,
                   num_devices=N_CORES)
    with tile.TileContext(nc) as tc:
        _emit_kernel(tc)
    nc.compile()
    _COMPILED["nc"] = nc
    return nc


def kernel(x, conv_weight, conv_bias):
    import ml_dtypes

    bf16 = ml_dtypes.bfloat16
    x = np.asarray(x, dtype=np.float32)
    conv_weight = np.asarray(conv_weight, dtype=np.float32)
    conv_bias = np.asarray(conv_bias, dtype=np.float32)

    xp = np.ascontiguousarray(
        x.transpose(0, 2, 1, 3, 4)).astype(bf16)  # [N, D, C, H, W]
    w_sb = _build_weight_blocks(conv_weight).astype(bf16)
    bias_sb = _build_bias128(conv_bias)
    ones_sb = _build_ones().astype(bf16)
    ones_bc_sb = _build_ones_bc().astype(bf16)

    nc = _compile()
    in_maps = []
    for i in range(N_CORES):
        xi = xp[NB * i:NB * (i + 1)].reshape(-1)
        xi = np.concatenate([xi, np.zeros(W, dtype=bf16)])
        in_maps.append({
            "x": np.ascontiguousarray(xi),
            "w": w_sb,
            "bias": bias_sb,
            "ones": ones_sb,
            "ones_bc": ones_bc_sb,
        })
    res = bass_utils.run_bass_kernel_spmd(
        nc, in_maps, core_ids=list(range(N_CORES)),
        trace=bool(int(os.environ.get("KERNEL_TRACE", "0"))),
    )
    _COMPILED["last_results"] = res
    # y_raw [NB, hh, cp, 128, 512]: partition = (k, c, co), free = (i, w);
    # h = 64*hh + 32*cp + 16*k + 4*c + i.  Untangle on the host.
    outs = []
    for i in range(N_CORES):
        yr = res.results[i]["y"].reshape(NB, 2, 2, 2, 4, CO, 4, W)
        #                                n  hh cp  k  c  co  i  w
        yi = yr.transpose(0, 5, 1, 2, 3, 4, 6, 7).reshape(NB, CO, 128, W)
        outs.append(yi[:, :, :HOUT, :WOUT])
    return np.ascontiguousarray(np.concatenate(outs, axis=0))


if __name__ == "__main__":
    _compile()
    print("build OK")


# revision 30
# speedup vs baseline: 1.0261x; 1.0261x over previous
"""Trainium2 Bass kernel for: Conv3d(3->16, k=3x3x3, VALID) + bias -> min over
depth -> softmax over channels.

Input  x: (16, 3, 32, 128, 128) f32   [N, C_in, D, H, W]
Weight w: (16, 3, 3, 3, 3) f32        [C_out, C_in, kD, kH, kW]
Bias   b: (16,) f32
Output  : (16, 16, 126, 126) f32      [N, C_out, H_out, W_out]

Data-parallel over batch: 2 batches per core x 8 cores. Per core:

  - x (bf16, host-transposed to [N, D, C, H, W] + one pad row) lives in quad
    tiles: strip r rows 0..11 = (d-window of 4) x (ci 3) for depth pair
    d0=8g+2r, rows 12..23 = the same planes shifted by one h-row.  Each
    strip loads with ONE 3-dim DMA (overlapping reads express the shift).
  - Conv as 16-way tile_position-packed bf16 matmuls, 6 accumulating passes
    per PSUM supertile: 3x K=12 (kh=2 at kw=0..2, base rows only, run first
    so they only wait on rows 0..11) then 3x K=24 (kh=0 and kh=1 fused via
    the shifted copy, kw=0..2).  PSUM supertile [128, 4*512]: partition =
    (c 4 chunks x delta x co), free = (pair r x 512 spatial).
  - Depth-min: g=0 supertile copied PSUM->SBUF on the Scalar engine
    (activation Copy); g=1..3 DVE tensor_tensor(min) against the running
    SBUF buffer.  DVE folds the 4 pair slots pairwise, then the delta
    halves are folded via merged shuffle DMAs (compute engines cannot
    address 16-partition offsets; one DMA per (k,c) moves both delta slots)
    plus one aligned DVE min.
  - Softmax over co: ACT exp (bias fused; min(y)+b == min(y+b)) -> bf16,
    PE ones-matmul for co-sums, DVE fast-approx reciprocal, PE ones-matmul
    partition broadcast, DVE multiply.  The result is dumped to DRAM in its
    native [128, 512] layout (one DMA per (n, hh, cp)); the host does the
    final (k,c,co,i,w) -> (co,h,w) untangling in numpy.
  - Each phase's softmax tail is emitted AFTER the next phase's conv
    matmuls (software pipelining) so the tail's PE ops never stall the
    conv pipeline; DMAs are spread across the Sync/Scalar HWDGE queues and
    the Pool SWDGE queue to dodge the ~2us-per-DMA queue serialization.
"""

import os
import sys

sys.path.insert(0, "/opt/trn_rl_repo")

import numpy as np

import concourse.bass as bass
import concourse.bacc as bacc
import concourse.tile as tile
import concourse.mybir as mybir
from concourse import bass_utils
from concourse.ap import AP

F32 = mybir.dt.float32
BF16 = mybir.dt.bfloat16

N_CORES = 8
NB = 2           # batches per core
CI = 3
D = 32
H = 128
W = 128
CO = 16
CHUNK = 512
NGRP = 4         # pair-quad groups; g<3: 4 pairs, g=3: 3 pairs
NCGL = 4         # chunk groups per h-half (each 4 col-tiles x 512)
HOUT = 126
WOUT = 126
PAD = 320
QF = 66 * W + PAD  # quad tile free size (worst case hh=0)
HW = H * W
XFLAT = NB * D * CI * HW + W  # one pad row so shifted reads never run off

_COMPILED = {}


def _pairs_in_group(g):
    return 4 if g < 3 else 3


def _build_weight_blocks(conv_weight):
    """[128, 192]: 6 col-sets of 32 (delta*16+co). Strip rows 0..11 hold the
    unshifted x copy, rows 12..23 the h+1-shifted copy. Sets 0-2 (K=24)
    contract kh=0 (rows 0-11) and kh=1 (rows 12-23) together at kw=set;
    sets 3-5 (K=12) contract kh=2 at kw=set-3. Value = w[co,ci,dl-delta,
    kh,kw], 0 outside kd range. Replicated per strip."""
    blk = np.zeros((24, 6, 32), dtype=np.float32)
    for dl in range(4):
        for ci in range(CI):
            row = dl * 3 + ci
            for delta in range(2):
                kd = dl - delta
                if not (0 <= kd <= 2):
                    continue
                cs = slice(delta * 16, (delta + 1) * 16)
                for kw in range(3):
                    blk[row, kw, cs] = conv_weight[:, ci, kd, 0, kw]
                    blk[row + 12, kw, cs] = conv_weight[:, ci, kd, 1, kw]
                    blk[row, 3 + kw, cs] = conv_weight[:, ci, kd, 2, kw]
    w_sb = np.zeros((128, 192), dtype=np.float32)
    for r in range(4):
        w_sb[32 * r:32 * r + 24, :] = blk.reshape(24, 192)
    return w_sb


def _build_ones():
    """[128, 8]: col k*4+c sums partitions {k*64 + c*16 + co : co}."""
    ones = np.zeros((128, 8), dtype=np.float32)
    for p in range(128):
        k, c = p // 64, (p % 64) // 16
        ones[p, k * 4 + c] = 1.0
    return ones


def _build_ones_bc():
    """[8, 128]: transpose of _build_ones -- broadcasts row j over its
    16-partition group."""
    return np.ascontiguousarray(_build_ones().T)


def _build_bias128(conv_bias):
    """[128, 1]: partition k*64 + c*16 + co -> bias[co]."""
    b = np.zeros((128, 1), dtype=np.float32)
    for p in range(128):
        b[p, 0] = conv_bias[p % 16]
    return b


def _emit_kernel(tc):
    nc = tc.nc
    x_t = nc.dram_tensor("x", [XFLAT], BF16, kind="ExternalInput")
    w_ap = nc.dram_tensor("w", [128, 192], BF16, kind="ExternalInput").ap()
    bias_ap = nc.dram_tensor("bias", [128, 1], F32, kind="ExternalInput").ap()
    ones_ap = nc.dram_tensor("ones", [128, 8], BF16, kind="ExternalInput").ap()
    ones_bc_ap = nc.dram_tensor("ones_bc", [8, 128], BF16,
                                kind="ExternalInput").ap()
    y_ap = nc.dram_tensor("y", [NB, 2, 2, 128, CHUNK], F32,
                          kind="ExternalOutput").ap()

    from contextlib import ExitStack

    MIN = mybir.AluOpType.min

    with ExitStack() as ctx:
        const_pool = ctx.enter_context(tc.tile_pool(name="const", bufs=1))
        in_pool = ctx.enter_context(tc.tile_pool(name="in", bufs=6))
        run_pool = ctx.enter_context(tc.tile_pool(name="run", bufs=3))
        sm_pool = ctx.enter_context(tc.tile_pool(name="sm", bufs=2))
        exp_pool = ctx.enter_context(tc.tile_pool(name="expp", bufs=2))
        psum_pool = ctx.enter_context(tc.tile_pool(name="ps", bufs=2,
                                                   space="PSUM"))

        w_sb = const_pool.tile([128, 192], BF16, tag="w")
        nc.scalar.dma_start(w_sb[:, :], w_ap[:, :])
        bias_sb = const_pool.tile([128, 1], F32, tag="bias")
        nc.scalar.dma_start(bias_sb[:, :], bias_ap[:, :])
        ones_sb = const_pool.tile([128, 8], BF16, tag="ones")
        nc.scalar.dma_start(ones_sb[:, :], ones_ap[:, :])
        ones_bc_sb = const_pool.tile([8, 128], BF16, tag="onesbc")
        nc.scalar.dma_start(ones_bc_sb[:, :], ones_bc_ap[:, :])


        def emit_tail(n, hh, running):
            # tail for phase (n, hh): fold pair slots (free dim, DVE), then
            # merged delta shuffles into t2 [128, 2*512] (partition p = 64k
            # + 16c + co, free slot = delta), one aligned min, softmax, and
            # a single native-layout dump per cp.  Runs one phase late.
            for cp in range(NCGL // 2):
                t2 = sm_pool.tile([128, 2 * CHUNK], BF16, tag="t2")
                for k in range(2):
                    cgl = 2 * cp + k
                    run = running[cgl]
                    a = sm_pool.tile([128, CHUNK], BF16, tag="a")
                    b = sm_pool.tile([128, CHUNK], BF16, tag="b")
                    nc.vector.tensor_tensor(
                        out=a[:, :], in0=run[:, 0:512],
                        in1=run[:, 512:1024], op=MIN)
                    nc.vector.tensor_tensor(
                        out=b[:, :], in0=run[:, 1024:1536],
                        in1=run[:, 1536:2048], op=MIN)
                    rm = sm_pool.tile([128, CHUNK], BF16, tag="rm")
                    nc.vector.tensor_tensor(
                        out=rm[:, :], in0=a[:, :], in1=b[:, :], op=MIN)
                    engs = (nc.sync, nc.scalar, nc.scalar)
                    for c in range(4):
                        base = 64 * k + 16 * c
                        for dlt in range(2):
                            q = ((2 * cp + k) * 4 + c) * 2 + dlt
                            engs[q % 2].dma_start(
                                t2[base:base + 16,
                                   CHUNK * dlt:CHUNK * (dlt + 1)],
                                rm[32 * c + 16 * dlt:
                                   32 * c + 16 * dlt + 16, :])
                sm = sm_pool.tile([128, CHUNK], BF16, tag="sm")
                nc.vector.tensor_tensor(
                    out=sm[:, :], in0=t2[:, 0:CHUNK],
                    in1=t2[:, CHUNK:2 * CHUNK], op=MIN)
                expt = exp_pool.tile([128, CHUNK], BF16, tag="exp")
                nc.scalar.activation(
                    expt[:, :], sm[:, :],
                    mybir.ActivationFunctionType.Exp,
                    bias=bias_sb[:, :], scale=1.0)
                esum = psum_pool.tile([8, CHUNK], F32, tag="big")
                nc.tensor.matmul(esum[:, :], lhsT=ones_sb[:, :],
                                 rhs=expt[:, :], start=True, stop=True)
                rec = sm_pool.tile([8, CHUNK], F32, tag="rec")
                nc.vector.reciprocal_approx_fast(rec[:, :], esum[:, :])
                rec_bf = sm_pool.tile([8, CHUNK], BF16, tag="recbf")
                nc.vector.tensor_copy(rec_bf[:, :], rec[:, :])
                # broadcast rec rows over their 16-co partition groups
                # via a K=8 ones matmul (PE does the partition fan-out)
                rb = psum_pool.tile([128, CHUNK], F32, tag="big")
                nc.tensor.matmul(rb[:, :], lhsT=ones_bc_sb[:, :],
                                 rhs=rec_bf[:, :], start=True, stop=True)
                soft = sm_pool.tile([128, CHUNK], F32, tag="soft")
                nc.vector.tensor_tensor(
                    out=soft[:, :], in0=rb[:, :], in1=expt[:, :],
                    op=mybir.AluOpType.mult)
                nc.scalar.dma_start(y_ap[n, hh, cp], soft[:, :])

        prev_phase = None
        for n in range(NB):
            for hh in range(2):
                h0 = 64 * hh
                hrows = 66 if hh == 0 else 64
                running = [run_pool.tile([128, 4 * CHUNK], BF16,
                                         name=f"run{cg}", tag=f"run{cg}")
                           for cg in range(NCGL)]
                for g in range(NGRP):
                    npairs = _pairs_in_group(g)
                    quad = in_pool.tile([128, QF], BF16, tag="quad")
                    nc.gpsimd.memset(quad[:, (hrows - 1) * W:QF], 0.0)
                    # One 3-dim DMA per strip loads rows 0..11 (x planes at
                    # h0) AND rows 12..23 (same planes at h0+1): dims =
                    # [h-shift j:2 x step W][plane dc:12 x step HW][inner].
                    # hh=1's shifted copy reads one garbage row past each
                    # plane; it only feeds discarded h=127 outputs, and the
                    # host pads x with one row so the last read stays in
                    # bounds.
                    inner = hrows * W
                    for r in range(npairs):
                        d0 = 8 * g + 2 * r
                        off = (n * D + d0) * CI * HW + h0 * W
                        src = AP(x_t, off, [[W, 2], [HW, 12], [1, inner]])
                        eng = nc.sync if r % 2 == 0 else nc.scalar
                        eng.dma_start(
                            quad[32 * r:32 * r + 24, 0:inner].rearrange(
                                "(j q) f -> j q f", j=2), src)
                    for cgl in range(NCGL):
                        ps = psum_pool.tile([128, 4 * CHUNK], F32, tag="big")
                        for s in (3, 4, 5, 0, 1, 2):
                            kw = s % 3
                            koff = kw if s < 3 else 2 * W + kw
                            kk = 24 if s < 3 else 12
                            for r in range(npairs):
                                for c in range(4):
                                    s0 = cgl * 2048 + c * CHUNK + koff
                                    nc.tensor.matmul(
                                        ps[32 * c:32 * c + 32,
                                           r * CHUNK:(r + 1) * CHUNK],
                                        lhsT=w_sb[32 * r:32 * r + kk,
                                                  s * 32:(s + 1) * 32],
                                        rhs=quad[32 * r:32 * r + kk,
                                                 s0:s0 + CHUNK],
                                        start=(s == 3),
                                        stop=(s == 2),
                                        tile_position=(32 * r, 32 * c),
                                        skip_group_check=True,
                                    )
                        width = npairs * CHUNK
                        if g == 0:
                            nc.scalar.activation(
                                running[cgl][:, 0:width], ps[:, 0:width],
                                mybir.ActivationFunctionType.Copy)
                        else:
                            nc.vector.tensor_tensor(
                                out=running[cgl][:, 0:width],
                                in0=ps[:, 0:width],
                                in1=running[cgl][:, 0:width],
                                op=MIN,
                            )
                if prev_phase is not None:
                    emit_tail(*prev_phase)
                prev_phase = (n, hh, running)
        emit_tail(*prev_phase)


def _compile():
    if "nc" in _COMPILED:
        return _COMPILED["nc"]
    nc = bacc.Bacc("TRN2", target_bir_lowering=False, debug=False,
                   num_devices=N_CORES)
    with tile.TileContext(nc) as tc:
        _emit_kernel(tc)
    nc.compile()
    _COMPILED["nc"] = nc
    return nc


def kernel(x, conv_weight, conv_bias):
    import ml_dtypes

    bf16 = ml_dtypes.bfloat16
    x = np.asarray(x, dtype=np.float32)
    conv_weight = np.asarray(conv_weight, dtype=np.float32)
    conv_bias = np.asarray(conv_bias, dtype=np.float32)

    xp = np.ascontiguousarray(
        x.transpose(0, 2, 1, 3, 4)).astype(bf16)  # [N, D, C, H, W]
    w_sb = _build_weight_blocks(conv_weight).astype(bf16)
    bias_sb = _build_bias128(conv_bias)
    ones_sb = _build_ones().astype(bf16)
    ones_bc_sb = _build_ones_bc().astype(bf16)

    nc = _compile()
    in_maps = []
    for i in range(N_CORES):
        xi = xp[NB * i:NB * (i + 1)].reshape(-1)
        xi = np.concatenate([xi, np.zeros(W, dtype=bf16)])
        in_maps.append({
            "x": np.ascontiguousarray(xi),
            "w": w_sb,
            "bias": bias_sb,
            "ones": ones_sb,
            "ones_bc": ones_bc_sb,
        })
    res = bass_utils.run_bass_kernel_spmd(
        nc, in_maps, core_ids=list(range(N_CORES)),
        trace=bool(int(os.environ.get("KERNEL_TRACE", "0"))),
    )
    _COMPILED["last_results"] = res
    # y_raw [NB, hh, cp, 128, 512]: partition = (k, c, co), free = (i, w);
    # h = 64*hh + 32*cp + 16*k + 4*c + i.  Untangle on the host.
    outs = []
    for i in range(N_CORES):
        yr = res.results[i]["y"].reshape(NB, 2, 2, 2, 4, CO, 4, W)
        #                                n  hh cp  k  c  co  i  w
        yi = yr.transpose(0, 5, 1, 2, 3, 4, 6, 7).reshape(NB, CO, 128, W)
        outs.append(yi[:, :, :HOUT, :WOUT])
    return np.ascontiguousarray(np.concatenate(outs, axis=0))


if __name__ == "__main__":
    _compile()
    print("build OK")
